# revision 1
# baseline (speedup 1.0000x reference)
"""MoE (dense-act-dense, top-4 of 8 experts) Trainium2 kernel.

Strategy (expert-parallel, host-side dispatch):
  - The forward combine weight is exactly 1.0 (straight-through gate trick in
    the reference), so out[n] = sum_{e in top4(n)} expert_e(x[n]).
  - Host computes the tiny gate matmul + top-4 routing (0.05% of FLOPs) and
    dispatches tokens: core e receives the tokens routed to expert e
    (capacity-padded), plus expert e's weights. This is the sharding step.
  - Each of the 8 cores runs a dense 2-layer MLP (relu between) on its tokens:
      h = relu(w1[e] @ x) ; y = w2[e] @ h
    as two chained fp32r GEMMs (fp32 data, FP22 multiply, fp32 accumulate).
  - Host scatter-adds per-expert outputs back (weight 1.0 per selection).

Per-core device layouts (everything pre-transposed on host for contiguous DMA):
  xT  [D, C] f32r : routed tokens, transposed
  w1t [D, H] f32r : w1[e].T
  w2t [H, O] f32r : w2[e].T
  yT  [O, C] f32  : expert output, transposed

Schedule notes:
  - Capacity is exact (max expert load, even-rounded), split into even tile
    widths in [256, 512]: fp32r requires even moving counts; measured per-mm
    spacing is ~(NT+32..44)cyc so per-token cost is flat for NT in 320..460.
  - Weights are DMAed in 128-wide column slices (separate tiles) so GEMM
    chains start as soon as their slice lands instead of after the full 8MB.
  - DMA emission order on the sync queue is hand-tuned: x0, w1 slices, x1,
    w2[0:8], x2, w2[8:16], x3, ... so the PE's program-order needs roughly
    track the FIFO queue's delivery order during the ~26MB startup stream.
  - GEMM2(t) is emitted one tile behind GEMM1(t+1) (depth-1 software
    pipeline) to give the PE GEMM1 work while w2 is still streaming in.
  - y drains: PSUM -> SBUF copy on vector, store DMA issued on scalar, so the
    sync queue (x + weights, latency-critical) is never blocked behind them.
"""

import numpy as np
from contextlib import ExitStack

import concourse.bass as bass
import concourse.tile as tile
from concourse import bacc, mybir
from concourse import bass_utils

F32 = mybir.dt.float32
F32R = mybir.dt.float32r
P = 128

TOP_K = 4
D, H, O, E = 2048, 1024, 2048, 8
_NC_CACHE = {}


def _tile_widths(C, target):
    """Split C tokens (padded to even) into even tiles of near-equal width in
    [256, 512]. Even widths are an fp32r ISA requirement; >=256 keeps fp32r at
    1 cycle/row; wider tiles amortize the fixed ~32-cycle per-matmul bubble."""
    C = max(C + (C % 2), 256)
    C2 = C // 2
    ntiles = min(-(-C // target), C2 // 128)
    base = C2 // ntiles
    rem = C2 - base * ntiles
    widths = [2 * (base + 1)] * rem + [2 * base] * (ntiles - rem)
    widths.sort(reverse=True)
    assert sum(widths) == C and all(256 <= w <= 512 and w % 2 == 0 for w in widths)
    return widths


def build_expert_kernel(C, target):
    """Per-core program: dense [C, D] @ [D, H] -> relu -> @ [H, O] in fp32r."""
    DC, HC, OC = D // P, H // P, O // P
    widths = _tile_widths(C, target)
    starts = [sum(widths[:i]) for i in range(len(widths))]
    NTILES = len(widths)
    NTMAX = max(widths)
    nc = bacc.Bacc("TRN2", target_bir_lowering=False, debug=False, num_devices=E)
    xT = nc.dram_tensor("xT", [D, C], F32R, kind="ExternalInput").ap()
    w1t = nc.dram_tensor("w1t", [D, H], F32R, kind="ExternalInput").ap()
    w2t = nc.dram_tensor("w2t", [H, O], F32R, kind="ExternalInput").ap()
    yT = nc.dram_tensor("yT", [O, C], F32, kind="ExternalOutput").ap()

    with tile.TileContext(nc) as tc, ExitStack() as ctx:
        wpool = ctx.enter_context(tc.tile_pool(name="w", bufs=1))
        xpool = ctx.enter_context(tc.tile_pool(name="x", bufs=2))
        hpool = ctx.enter_context(tc.tile_pool(name="h", bufs=2))
        ypool = ctx.enter_context(tc.tile_pool(name="y", bufs=4))
        ps1 = ctx.enter_context(tc.tile_pool(name="ps1", bufs=2, space="PSUM"))
        ps2 = ctx.enter_context(tc.tile_pool(name="ps2", bufs=4, space="PSUM"))

        x_tiles = {}

        def dma_x(t):
            w_t = widths[t]
            x_t = xpool.tile([P, DC, NTMAX], F32R, name="x_t")[:, :, :w_t]
            nc.sync.dma_start(
                x_t[:],
                xT[:, starts[t]:starts[t] + w_t].rearrange("(dc p) n -> p dc n", p=P),
            )
            x_tiles[t] = x_t

        # --- startup DMA stream, hand-ordered for the FIFO queue ---
        dma_x(0)
        w1s = []
        for hc in range(HC):
            w = wpool.tile([P, DC, P], F32R, name=f"w1s{hc}")
            nc.sync.dma_start(
                w[:],
                w1t[:, hc * P:(hc + 1) * P].rearrange("(dc p) h -> p dc h", p=P),
            )
            w1s.append(w)
        if NTILES > 1:
            dma_x(1)
        w2s = []

        def dma_w2(oc):
            w = wpool.tile([P, HC, P], F32R, name=f"w2s{oc}")
            nc.sync.dma_start(
                w[:],
                w2t[:, oc * P:(oc + 1) * P].rearrange("(hc p) o -> p hc o", p=P),
            )
            w2s.append(w)

        for oc in range(OC // 2):
            dma_w2(oc)

        def gemm1(t):
            w_t = widths[t]
            x_t = x_tiles.pop(t)
            h_t = hpool.tile([P, HC, NTMAX], F32R, name="h_t")[:, :, :w_t]
            for hc in range(HC):
                ph = ps1.tile([P, NTMAX], F32, name="ph")[:, :w_t]
                for dc in range(DC):
                    nc.tensor.matmul(
                        ph[:], w1s[hc][:, dc, :], x_t[:, dc, :],
                        start=(dc == 0), stop=(dc == DC - 1),
                    )
                nc.scalar.activation(
                    h_t[:, hc, :], ph[:], mybir.ActivationFunctionType.Relu
                )
            return h_t

        def gemm2(t, h_t):
            w_t = widths[t]
            for oc in range(OC):
                po = ps2.tile([P, NTMAX], F32, name="po")[:, :w_t]
                for hc in range(HC):
                    nc.tensor.matmul(
                        po[:], w2s[oc][:, hc, :], h_t[:, hc, :],
                        start=(hc == 0), stop=(hc == HC - 1),
                    )
                y_t = ypool.tile([P, NTMAX], F32, name="y_t")[:, :w_t]
                nc.vector.tensor_copy(y_t[:], po[:])
                nc.scalar.dma_start(
                    yT[oc * P:(oc + 1) * P, starts[t]:starts[t] + w_t], y_t[:]
                )

        # --- depth-1 software-pipelined main loop: GEMM2 runs one tile
        # behind GEMM1 so the PE has work while w2 streams in at startup ---
        h_tiles = {}
        for t in range(NTILES):
            if t + 1 < NTILES and t >= 1:
                dma_x(t + 1)
            h_tiles[t] = gemm1(t)
            if t == 1:
                for oc in range(OC // 2, OC):
                    dma_w2(oc)
            if t >= 1:
                gemm2(t - 1, h_tiles.pop(t - 1))
        gemm2(NTILES - 1, h_tiles.pop(NTILES - 1))
    nc.compile()
    return nc


def _route(xt, wg):
    """Host-side gate + top-4. Gap between 4th/5th gate values is ~3e-5 for
    this distribution, far above fp32 matmul noise, so fp32 reproduces the
    reference top-k set exactly."""
    gate = xt @ wg  # [N, E] fp32
    top4 = np.argpartition(-gate, TOP_K - 1, axis=1)[:, :TOP_K]  # set, unordered
    return top4


def kernel(x, wg, w1, w2, _want_results=False, _run_kwargs=None):
    x = np.asarray(x, dtype=np.float32)
    wg = np.asarray(wg, dtype=np.float32)
    w1 = np.asarray(w1, dtype=np.float32)
    w2 = np.asarray(w2, dtype=np.float32)
    B, S, Dx = x.shape
    N = B * S
    xt = np.ascontiguousarray(x.reshape(N, Dx))
    top4 = _route(xt, wg)

    # token lists per expert
    sel = np.zeros((N, E), dtype=bool)
    np.put_along_axis(sel, top4, True, axis=1)
    tokens = [np.nonzero(sel[:, e])[0] for e in range(E)]
    counts = np.array([len(t) for t in tokens])
    CAP = max(int(counts.max()), 256)
    CAP += CAP % 2

    if CAP not in _NC_CACHE:
        # Wider tiles amortize the per-matmul bubble best, but the widest
        # config cuts SBUF very close — fall back to narrower tiles if the
        # allocator rejects it.
        last_err = None
        for target in (384, 352, 320):
            try:
                _NC_CACHE[CAP] = build_expert_kernel(CAP, target)
                break
            except ValueError as err:  # SBUF pool allocation failure
                last_err = err
        else:
            raise last_err
    nc = _NC_CACHE[CAP]

    in_maps = []
    for e in range(E):
        xe = np.zeros((CAP, Dx), dtype=np.float32)
        xe[:counts[e]] = xt[tokens[e]]
        in_maps.append({
            "xT": np.ascontiguousarray(xe.T),
            "w1t": np.ascontiguousarray(w1[e].T),
            "w2t": np.ascontiguousarray(w2[e].T),
        })

    res = bass_utils.run_bass_kernel_spmd(
        nc, in_maps, core_ids=list(range(E)), **(_run_kwargs or {})
    )

    out = np.zeros((N, O), dtype=np.float32)
    for e in range(E):
        out[tokens[e]] += res.results[e]["yT"].T[:counts[e]]
    out = out.reshape(B, S, O)
    if _want_results:
        return out, res
    return out



# revision 2
# speedup vs baseline: 1.0995x; 1.0995x over previous
"""MoE (dense-act-dense, top-4 of 8 experts) Trainium2 kernel.

Strategy (expert-parallel, host-side dispatch):
  - The forward combine weight is exactly 1.0 (straight-through gate trick in
    the reference), so out[n] = sum_{e in top4(n)} expert_e(x[n]).
  - Host computes the tiny gate matmul + top-4 routing (0.05% of FLOPs) and
    dispatches tokens: core e receives the tokens routed to expert e
    (capacity-padded), plus expert e's weights. This is the sharding step.
  - Each of the 8 cores runs a dense 2-layer MLP (relu between) on its tokens:
      h = relu(w1[e] @ x) ; y = w2[e] @ h
    as two chained GEMMs in bf16 (fp32 PSUM accumulate).
  - Host scatter-adds per-expert outputs back (weight 1.0 per selection).

Why bf16 (not fp32r): trace analysis showed the fp32r kernel was
LDWEIGHTS-bound — a 128x128 fp32r stationary load takes ~226ns, longer than
the matmul itself at NT=384 (~160ns), so the PE cadence was ~225ns/matmul.
bf16 halves the stationary load (~110ns), making the matmul compute the
binding constraint again, and also halves all DMA traffic (startup latency,
drain, chip-wide HBM contention). bf16 end-to-end rel err ~3e-3, far below
the 2e-2 gate (routing stays exact: gate+top4 are computed on host in fp32).

Per-core device layouts (everything pre-transposed on host for contiguous DMA):
  xT  [D, C] bf16 : routed tokens, transposed
  w1t [D, H] bf16 : w1[e].T
  w2t [H, O] bf16 : w2[e].T
  yT  [O, C] bf16 : expert output, transposed

Schedule notes:
  - Capacity is exact (max expert load, even-rounded), split into even tile
    widths in [256, 512] (PSUM bank caps a matmul's moving dim at 512 fp32).
  - Weights are DMAed in 128-wide column slices (separate tiles) so GEMM
    chains start as soon as their slice lands instead of after the full 8MB.
  - DMA emission order on the sync queue is hand-tuned: x0, w1 slices, x1,
    w2[0:8], x2, w2[8:16], x3, ... so the PE's program-order needs roughly
    track the FIFO queue's delivery order during the startup stream.
  - GEMM2(t) is emitted one tile behind GEMM1(t+1) (depth-1 software
    pipeline) to give the PE GEMM1 work while w2 is still streaming in.
  - y drains: PSUM -> SBUF copy on vector (cast to bf16), store DMA issued on
    scalar, so the sync queue (x + weights, latency-critical) is never
    blocked behind them.
"""

import numpy as np
import ml_dtypes
from contextlib import ExitStack

import concourse.bass as bass
import concourse.tile as tile
from concourse import bacc, mybir
from concourse import bass_utils

F32 = mybir.dt.float32
BF16 = mybir.dt.bfloat16
P = 128

TOP_K = 4
D, H, O, E = 2048, 1024, 2048, 8
_NC_CACHE = {}


def _tile_widths(C, target):
    """Split C tokens (padded to even) into even tiles of near-equal width in
    [256, 512]. 512 is the PSUM bank cap on a single matmul's moving dim;
    wider tiles amortize the fixed per-matmul issue bubble."""
    C = max(C + (C % 2), 256)
    C2 = C // 2
    ntiles = min(-(-C // target), C2 // 128)
    base = C2 // ntiles
    rem = C2 - base * ntiles
    widths = [2 * (base + 1)] * rem + [2 * base] * (ntiles - rem)
    widths.sort(reverse=True)
    assert sum(widths) == C and all(256 <= w <= 512 and w % 2 == 0 for w in widths)
    return widths


def build_expert_kernel(C, target):
    """Per-core program: dense [C, D] @ [D, H] -> relu -> @ [H, O] in bf16."""
    DC, HC, OC = D // P, H // P, O // P
    widths = _tile_widths(C, target)
    starts = [sum(widths[:i]) for i in range(len(widths))]
    NTILES = len(widths)
    NTMAX = max(widths)
    nc = bacc.Bacc("TRN2", target_bir_lowering=False, debug=False, num_devices=E)
    xT = nc.dram_tensor("xT", [D, C], BF16, kind="ExternalInput").ap()
    w1t = nc.dram_tensor("w1t", [D, H], BF16, kind="ExternalInput").ap()
    w2t = nc.dram_tensor("w2t", [H, O], BF16, kind="ExternalInput").ap()
    yT = nc.dram_tensor("yT", [O, C], BF16, kind="ExternalOutput").ap()

    with tile.TileContext(nc) as tc, ExitStack() as ctx:
        wpool = ctx.enter_context(tc.tile_pool(name="w", bufs=1))
        xpool = ctx.enter_context(tc.tile_pool(name="x", bufs=2))
        hpool = ctx.enter_context(tc.tile_pool(name="h", bufs=2))
        ypool = ctx.enter_context(tc.tile_pool(name="y", bufs=4))
        ps1 = ctx.enter_context(tc.tile_pool(name="ps1", bufs=2, space="PSUM"))
        ps2 = ctx.enter_context(tc.tile_pool(name="ps2", bufs=4, space="PSUM"))

        x_tiles = {}

        def dma_x(t):
            w_t = widths[t]
            x_t = xpool.tile([P, DC, NTMAX], BF16, name="x_t")[:, :, :w_t]
            nc.sync.dma_start(
                x_t[:],
                xT[:, starts[t]:starts[t] + w_t].rearrange("(dc p) n -> p dc n", p=P),
            )
            x_tiles[t] = x_t

        # --- startup DMA stream, hand-ordered for the FIFO queue ---
        dma_x(0)
        w1s = []
        for hc in range(HC):
            w = wpool.tile([P, DC, P], BF16, name=f"w1s{hc}")
            nc.sync.dma_start(
                w[:],
                w1t[:, hc * P:(hc + 1) * P].rearrange("(dc p) h -> p dc h", p=P),
            )
            w1s.append(w)
        if NTILES > 1:
            dma_x(1)
        w2s = []

        def dma_w2(oc):
            w = wpool.tile([P, HC, P], BF16, name=f"w2s{oc}")
            nc.sync.dma_start(
                w[:],
                w2t[:, oc * P:(oc + 1) * P].rearrange("(hc p) o -> p hc o", p=P),
            )
            w2s.append(w)

        for oc in range(OC // 2):
            dma_w2(oc)

        def gemm1(t):
            w_t = widths[t]
            x_t = x_tiles.pop(t)
            h_t = hpool.tile([P, HC, NTMAX], BF16, name="h_t")[:, :, :w_t]
            for hc in range(HC):
                ph = ps1.tile([P, NTMAX], F32, name="ph")[:, :w_t]
                for dc in range(DC):
                    nc.tensor.matmul(
                        ph[:], w1s[hc][:, dc, :], x_t[:, dc, :],
                        start=(dc == 0), stop=(dc == DC - 1),
                    )
                nc.scalar.activation(
                    h_t[:, hc, :], ph[:], mybir.ActivationFunctionType.Relu
                )
            return h_t

        def gemm2(t, h_t):
            w_t = widths[t]
            for oc in range(OC):
                po = ps2.tile([P, NTMAX], F32, name="po")[:, :w_t]
                for hc in range(HC):
                    nc.tensor.matmul(
                        po[:], w2s[oc][:, hc, :], h_t[:, hc, :],
                        start=(hc == 0), stop=(hc == HC - 1),
                    )
                y_t = ypool.tile([P, NTMAX], BF16, name="y_t")[:, :w_t]
                nc.vector.tensor_copy(y_t[:], po[:])
                nc.scalar.dma_start(
                    yT[oc * P:(oc + 1) * P, starts[t]:starts[t] + w_t], y_t[:]
                )

        # --- depth-1 software-pipelined main loop: GEMM2 runs one tile
        # behind GEMM1 so the PE has work while w2 streams in at startup ---
        h_tiles = {}
        for t in range(NTILES):
            if t + 1 < NTILES and t >= 1:
                dma_x(t + 1)
            h_tiles[t] = gemm1(t)
            if t == 1:
                for oc in range(OC // 2, OC):
                    dma_w2(oc)
            if t >= 1:
                gemm2(t - 1, h_tiles.pop(t - 1))
        gemm2(NTILES - 1, h_tiles.pop(NTILES - 1))
    nc.compile()
    return nc


def _route(xt, wg):
    """Host-side gate + top-4. Gap between 4th/5th gate values is ~3e-5 for
    this distribution, far above fp32 matmul noise, so fp32 reproduces the
    reference top-k set exactly."""
    gate = xt @ wg  # [N, E] fp32
    top4 = np.argpartition(-gate, TOP_K - 1, axis=1)[:, :TOP_K]  # set, unordered
    return top4


def kernel(x, wg, w1, w2, _want_results=False, _run_kwargs=None):
    x = np.asarray(x, dtype=np.float32)
    wg = np.asarray(wg, dtype=np.float32)
    w1 = np.asarray(w1, dtype=np.float32)
    w2 = np.asarray(w2, dtype=np.float32)
    B, S, Dx = x.shape
    N = B * S
    xt = np.ascontiguousarray(x.reshape(N, Dx))
    top4 = _route(xt, wg)

    # token lists per expert
    sel = np.zeros((N, E), dtype=bool)
    np.put_along_axis(sel, top4, True, axis=1)
    tokens = [np.nonzero(sel[:, e])[0] for e in range(E)]
    counts = np.array([len(t) for t in tokens])
    CAP = max(int(counts.max()), 256)
    CAP += CAP % 2

    if CAP not in _NC_CACHE:
        last_err = None
        for target in (512, 448, 384):
            try:
                _NC_CACHE[CAP] = build_expert_kernel(CAP, target)
                break
            except ValueError as err:  # SBUF pool allocation failure
                last_err = err
        else:
            raise last_err
    nc = _NC_CACHE[CAP]

    xtb = xt.astype(ml_dtypes.bfloat16)
    in_maps = []
    for e in range(E):
        xe = np.zeros((CAP, Dx), dtype=ml_dtypes.bfloat16)
        xe[:counts[e]] = xtb[tokens[e]]
        in_maps.append({
            "xT": np.ascontiguousarray(xe.T),
            "w1t": np.ascontiguousarray(w1[e].T.astype(ml_dtypes.bfloat16)),
            "w2t": np.ascontiguousarray(w2[e].T.astype(ml_dtypes.bfloat16)),
        })

    res = bass_utils.run_bass_kernel_spmd(
        nc, in_maps, core_ids=list(range(E)), **(_run_kwargs or {})
    )

    out = np.zeros((N, O), dtype=np.float32)
    for e in range(E):
        out[tokens[e]] += res.results[e]["yT"].T[:counts[e]].astype(np.float32)
    out = out.reshape(B, S, O)
    if _want_results:
        return out, res
    return out


# revision 5
# speedup vs baseline: 1.3139x; 1.1950x over previous
"""MoE (dense-act-dense, top-4 of 8 experts) Trainium2 kernel.

Strategy (expert-parallel, host-side dispatch):
  - The forward combine weight is exactly 1.0 (straight-through gate trick in
    the reference), so out[n] = sum_{e in top4(n)} expert_e(x[n]).
  - Host computes the tiny gate matmul + top-4 routing (0.05% of FLOPs) and
    dispatches tokens: core e receives the tokens routed to expert e
    (capacity-padded), plus expert e's weights. This is the sharding step.
  - Each of the 8 cores runs a dense 2-layer MLP (relu between) on its tokens:
      h = relu(w1[e] @ x) ; y = w2[e] @ h
    as two chained GEMMs in bf16 (fp32 PSUM accumulate).
  - Host scatter-adds per-expert outputs back (weight 1.0 per selection).

Why bf16 (not fp32r): trace analysis showed the fp32r kernel was
LDWEIGHTS-bound — a 128x128 fp32r stationary load takes ~226ns, longer than
the matmul itself at NT=384 (~160ns), so the PE cadence was ~225ns/matmul.
bf16 halves the stationary load (~110ns), making the matmul compute the
binding constraint again, and also halves all DMA traffic (startup latency,
drain, chip-wide HBM contention). bf16 end-to-end rel err ~3e-3, far below
the 2e-2 gate (routing stays exact: gate+top4 are computed on host in fp32).

Per-core device layouts (everything pre-transposed on host for contiguous DMA):
  xT  [D, C] bf16 : routed tokens, transposed
  w1t [D, H] bf16 : w1[e].T
  w2t [H, O] bf16 : w2[e].T
  yT  [O, C] bf16 : expert output, transposed

Schedule notes:
  - Capacity is exact (max expert load, even-rounded), split into even tile
    widths in [256, 512] (PSUM bank caps a matmul's moving dim at 512 fp32).
  - Weights are DMAed in 128-wide column slices (separate tiles) so GEMM
    chains start as soon as their slice lands instead of after the full 8MB.
  - DMA emission order on the sync queue is hand-tuned: x0, w1 slices, x1,
    w2[0:8], x2, w2[8:16], x3, ... so the PE's program-order needs roughly
    track the FIFO queue's delivery order during the startup stream.
  - GEMM2(t) is emitted one tile behind GEMM1(t+1) (depth-1 software
    pipeline) to give the PE GEMM1 work while w2 is still streaming in.
  - y drains: PSUM -> SBUF copy on vector (cast to bf16), store DMA issued on
    scalar, so the sync queue (x + weights, latency-critical) is never
    blocked behind them.
"""

import numpy as np
import ml_dtypes
from contextlib import ExitStack

import concourse.bass as bass
import concourse.tile as tile
from concourse import bacc, mybir
from concourse import bass_utils

F32 = mybir.dt.float32
BF16 = mybir.dt.bfloat16
P = 128

TOP_K = 4
D, H, O, E = 2048, 1024, 2048, 8
_NC_CACHE = {}


def _tile_widths(C, target):
    """Split C tokens (padded to even) into even tiles of near-equal width in
    [256, 512]. 512 is the PSUM bank cap on a single matmul's moving dim;
    wider tiles amortize the fixed per-matmul issue bubble."""
    C = max(C + (C % 2), 256)
    C2 = C // 2
    ntiles = min(-(-C // target), C2 // 128)
    base = C2 // ntiles
    rem = C2 - base * ntiles
    widths = [2 * (base + 1)] * rem + [2 * base] * (ntiles - rem)
    widths.sort(reverse=True)
    assert sum(widths) == C and all(256 <= w <= 512 and w % 2 == 0 for w in widths)
    return widths


def build_expert_kernel(C, target):
    """Per-core program: dense [C, D] @ [D, H] -> relu -> @ [H, O] in bf16."""
    DC, HC, OC = D // P, H // P, O // P
    widths = _tile_widths(C, target)
    starts = [sum(widths[:i]) for i in range(len(widths))]
    NTILES = len(widths)
    NTMAX = max(widths)
    nc = bacc.Bacc("TRN2", target_bir_lowering=False, debug=False, num_devices=E)
    xT = nc.dram_tensor("xT", [D, C], BF16, kind="ExternalInput").ap()
    w1t = nc.dram_tensor("w1t", [D, H], BF16, kind="ExternalInput").ap()
    w2t = nc.dram_tensor("w2t", [H, O], BF16, kind="ExternalInput").ap()
    yT = nc.dram_tensor("yT", [O, C], BF16, kind="ExternalOutput").ap()

    with tile.TileContext(nc) as tc, ExitStack() as ctx:
        wpool = ctx.enter_context(tc.tile_pool(name="w", bufs=1))
        xpool = ctx.enter_context(tc.tile_pool(name="x", bufs=4))
        hpool = ctx.enter_context(tc.tile_pool(name="h", bufs=4))
        ypool = ctx.enter_context(tc.tile_pool(name="y", bufs=4))
        ps1 = ctx.enter_context(tc.tile_pool(name="ps1", bufs=4, space="PSUM"))
        ps2 = ctx.enter_context(tc.tile_pool(name="ps2", bufs=4, space="PSUM"))

        x_tiles = {}

        def dma_x(t):
            w_t = widths[t]
            x_t = xpool.tile([P, DC, NTMAX], BF16, name="x_t")[:, :, :w_t]
            nc.sync.dma_start(
                x_t[:],
                xT[:, starts[t]:starts[t] + w_t].rearrange("(dc p) n -> p dc n", p=P),
            )
            x_tiles[t] = x_t

        # --- startup DMA stream, hand-ordered for the FIFO queue ---
        dma_x(0)
        w1s = []
        for hc in range(HC):
            w = wpool.tile([P, DC, P], BF16, name=f"w1s{hc}")
            nc.sync.dma_start(
                w[:],
                w1t[:, hc * P:(hc + 1) * P].rearrange("(dc p) h -> p dc h", p=P),
            )
            w1s.append(w)
        if NTILES > 1:
            dma_x(1)
        if NTILES > 2:
            dma_x(2)
        w2s = []

        def dma_w2(oc):
            w = wpool.tile([P, HC, P], BF16, name=f"w2s{oc}")
            nc.sync.dma_start(
                w[:],
                w2t[:, oc * P:(oc + 1) * P].rearrange("(hc p) o -> p hc o", p=P),
            )
            w2s.append(w)

        for oc in range(OC // 2):
            dma_w2(oc)

        def gemm1(ts):
            """Fused GEMM1 over a group of token tiles: the dc loop is outer,
            the tile loop inner, so consecutive matmuls share the stationary
            w1 block (amortizes the PE weight-swap bubble)."""
            hs = {}
            for t in ts:
                w_t = widths[t]
                hs[t] = hpool.tile([P, HC, NTMAX], BF16, name="h_t")[:, :, :w_t]
            phs = {}
            for hc in range(HC):
                for t in ts:
                    phs[t] = ps1.tile([P, NTMAX], F32, name="ph")[:, :widths[t]]
                for dc in range(DC):
                    for t in ts:
                        nc.tensor.matmul(
                            phs[t][:], w1s[hc][:, dc, :], x_tiles[t][:, dc, :],
                            start=(dc == 0), stop=(dc == DC - 1),
                        )
                for t in ts:
                    nc.scalar.activation(
                        hs[t][:, hc, :], phs[t][:],
                        mybir.ActivationFunctionType.Relu,
                    )
            for t in ts:
                x_tiles.pop(t)
                h_tiles[t] = hs[t]

        def gemm2(ts):
            """Fused GEMM2 over a group of token tiles (same-stationary)."""
            hs = {t: h_tiles.pop(t) for t in ts}
            pos = {}
            for oc in range(OC):
                for t in ts:
                    pos[t] = ps2.tile([P, NTMAX], F32, name="po")[:, :widths[t]]
                for hc in range(HC):
                    for t in ts:
                        nc.tensor.matmul(
                            pos[t][:], w2s[oc][:, hc, :], hs[t][:, hc, :],
                            start=(hc == 0), stop=(hc == HC - 1),
                        )
                for t in ts:
                    w_t = widths[t]
                    y_t = ypool.tile([P, NTMAX], BF16, name="y_t")[:, :w_t]
                    nc.vector.tensor_copy(y_t[:], pos[t][:])
                    nc.scalar.dma_start(
                        yT[oc * P:(oc + 1) * P, starts[t]:starts[t] + w_t],
                        y_t[:],
                    )

        # --- group tiles: tile 0 alone (starts as soon as x0+w1s[0] land),
        # the rest in pairs; GEMM2 runs one group behind GEMM1 so the PE has
        # GEMM1 work while w2 streams in ---
        groups = [(0,)] + [
            tuple(range(t, min(t + 2, NTILES))) for t in range(1, NTILES, 2)
        ]
        h_tiles = {}
        for gi, g in enumerate(groups):
            if gi + 1 < len(groups) and gi >= 1:
                for t in groups[gi + 1]:
                    dma_x(t)
            gemm1(g)
            if gi == 1:
                for oc in range(OC // 2, OC):
                    dma_w2(oc)
            if gi >= 1:
                gemm2(groups[gi - 1])
        gemm2(groups[-1])
    nc.compile()
    return nc


def _route(xt, wg):
    """Host-side gate + top-4. Gap between 4th/5th gate values is ~3e-5 for
    this distribution, far above fp32 matmul noise, so fp32 reproduces the
    reference top-k set exactly."""
    gate = xt @ wg  # [N, E] fp32
    top4 = np.argpartition(-gate, TOP_K - 1, axis=1)[:, :TOP_K]  # set, unordered
    return top4


def kernel(x, wg, w1, w2, _want_results=False, _run_kwargs=None):
    x = np.asarray(x, dtype=np.float32)
    wg = np.asarray(wg, dtype=np.float32)
    w1 = np.asarray(w1, dtype=np.float32)
    w2 = np.asarray(w2, dtype=np.float32)
    B, S, Dx = x.shape
    N = B * S
    xt = np.ascontiguousarray(x.reshape(N, Dx))
    top4 = _route(xt, wg)

    # token lists per expert
    sel = np.zeros((N, E), dtype=bool)
    np.put_along_axis(sel, top4, True, axis=1)
    tokens = [np.nonzero(sel[:, e])[0] for e in range(E)]
    counts = np.array([len(t) for t in tokens])
    CAP = max(int(counts.max()), 256)
    CAP += CAP % 2

    if CAP not in _NC_CACHE:
        last_err = None
        for target in (512, 448, 384):
            try:
                _NC_CACHE[CAP] = build_expert_kernel(CAP, target)
                break
            except ValueError as err:  # SBUF pool allocation failure
                last_err = err
        else:
            raise last_err
    nc = _NC_CACHE[CAP]

    xtb = xt.astype(ml_dtypes.bfloat16)
    in_maps = []
    for e in range(E):
        xe = np.zeros((CAP, Dx), dtype=ml_dtypes.bfloat16)
        xe[:counts[e]] = xtb[tokens[e]]
        in_maps.append({
            "xT": np.ascontiguousarray(xe.T),
            "w1t": np.ascontiguousarray(w1[e].T.astype(ml_dtypes.bfloat16)),
            "w2t": np.ascontiguousarray(w2[e].T.astype(ml_dtypes.bfloat16)),
        })

    res = bass_utils.run_bass_kernel_spmd(
        nc, in_maps, core_ids=list(range(E)), **(_run_kwargs or {})
    )

    out = np.zeros((N, O), dtype=np.float32)
    for e in range(E):
        out[tokens[e]] += res.results[e]["yT"].T[:counts[e]].astype(np.float32)
    out = out.reshape(B, S, O)
    if _want_results:
        return out, res
    return out
